# revision 61
# baseline (speedup 1.0000x reference)
"""BlockGRU Trainium2 kernel — fp8 DoubleRow edition.

Block-diagonal GRU cell: 8 independent blocks (block_size 256), batch 2048,
input_dim 1024. Sharded one block per NeuronCore (8 cores).

All matmuls run in fp8 e4m3 with MatmulPerfMode.DoubleRow (0.5 cycles per
output row, two 128-deep k-tiles per instruction -> 4x the fp16 PE rate in
the cost model). Precision is recovered with residual ("split hi/lo")
correction terms, applied only where the end-to-end error needs them:

  gi = x8@W8 (+ xr8@W8 + x8@WR8 on the n gate)
  gh = h8@Wh8

where x8 = e4m3(16*x), xr8 = e4m3(16*x - x8), W8 = e4m3(1024*W),
WR8 = e4m3(1024*W - W8), h8 = e4m3(16*h). All PSUM pre-activations share
one scale 16384, folded into the ScalarE activation `scale` operand.
Measured end-to-end rel-L2 error vs the fp32 reference: 1.632e-2
(gate: 2e-2; inputs are a fixed seed, so this is deterministic — the
numpy predictor in the dev transcript matches the kernel to ~1e-5).
Output is fp16, upcast on the host.

Elementwise pipeline (per state-tile): r and z sigmoids and the n tanh on
ScalarE (PSUM reads, fp16 outputs); the r-gating stt and the i_n add on
VectorE in fp32 (PSUM operands, magnitude up to ~1e5 — fp16 would
overflow); the output combine out = n + z*(h16 - n) in all-fp16
tensor_tensor ops, which the DVE cost model runs at 2x (2-byte packed
operands). h16 rides the per-chunk input stream. For mid chunks the
t0 combine runs on VectorE and the t1 combine on the otherwise-idle Pool
in the equivalent zc*n + z*h form (zc = sigmoid(-pre), so only two Pool
ops remain once the tanh lands); engine wait queues release in FIFO
order, so long-parked ops must never sit ahead of soon-needed ones on the
same queue. Output DMAs ride the otherwise-idle SP queue.

PE order per chunk completes psums in elementwise consumption order
(r -> hn -> z -> i_n) and starts the n-gate T1 late so the previous
chunk's b2 has drained PSUM; the last chunk inverts to r -> z-T1 -> hn ->
i_n -> z-tails so only sigmoid -> m -> out remain after the final matmul.
Input DMAs are merged and ordered so arrival tracks first-use (the serial
HWDGE costs ~625ns per DMA), and the warm-up is sized to keep the PE busy
until the first weights arrive (an idle PE resets the p-state ramp).
"""

import sys

if "/opt/trn_rl_repo" not in sys.path:
    sys.path.insert(0, "/opt/trn_rl_repo")

import numpy as np
import ml_dtypes

INPUT_DIM = 1024
HIDDEN_DIM = 2048
NUM_BLOCKS = 8
BS = HIDDEN_DIM // NUM_BLOCKS  # 256
G3 = 3 * BS                    # 768
BATCH = 2048
CHUNKS = [256, 512, 512, 512, 256]
JX = 4                         # input-side k-pairs (8 k-tiles, DoubleRow'd)
ST = 2                         # state partition-tiles per block
SX = 16.0                      # x / h quantization scale
SW = 1024.0                    # weight quantization scale
Q = SX * SW                    # psum pre-activation scale
INV = 1.0 / Q
MISC = 2 * G3 + JX * 2 * BS + 40   # wh | wr | bias bytes per partition

E4 = ml_dtypes.float8_e4m3

_cached = None


def _build():
    import concourse.tile as tile
    import concourse.mybir as mybir
    from concourse import bacc

    f32 = mybir.dt.float32
    f16 = mybir.dt.float16
    f8 = mybir.dt.float8e4
    u8 = mybir.dt.uint8
    ALU = mybir.AluOpType
    ACT = mybir.ActivationFunctionType
    DR = mybir.MatmulPerfMode.DoubleRow

    nc = bacc.Bacc("TRN2", target_bir_lowering=False, debug=False, num_devices=8)

    # DRAM tensors. Free-dim layouts are pre-packed on the host so every DMA
    # lands >=512B-contiguous runs (fp8 would otherwise pay the 2x
    # small-element DMA penalty). sA carries per-chunk
    # [x8 (8cw) | h8 (2cw) | h16 (4cw bytes)] blocks; xr8 streams
    # separately (it is consumed mid-chunk).
    sAd = nc.dram_tensor("sA", [128, 14 * BATCH], u8, kind="ExternalInput")
    xr8d = nc.dram_tensor("xr8", [128, 8 * BATCH], f8, kind="ExternalInput")
    w8d = nc.dram_tensor("w8", [128, JX * 2 * G3], f8, kind="ExternalInput")
    miscd = nc.dram_tensor("misc", [128, MISC], u8, kind="ExternalInput")
    oT = nc.dram_tensor("oT", [BS, BATCH], f16, kind="ExternalOutput")

    with tile.TileContext(nc) as tc:
        with (
            tc.tile_pool(name="const", bufs=1) as cp,
            tc.tile_pool(name="xin", bufs=1) as xp,
            tc.tile_pool(name="xrin", bufs=1) as xrp,
            tc.tile_pool(name="gates", bufs=2) as gp,
            tc.tile_pool(name="outs", bufs=1) as op,
            tc.tile_pool(name="psum", bufs=1, space="PSUM") as pp,
        ):
            # PE warm-up: matmuls on a zeroed tile while the prefill DMA
            # runs. Sized to keep the PE busy until the first weights/x
            # arrive (~5.8us): an idle PE resets the p-state ramp and the
            # first 3us of real matmuls would run at half clock.
            wu = cp.tile([128, 512], f16, tag="wu")
            nc.vector.memset(wu[:], 0.0)
            pdummy = pp.tile([128, 512], f32, tag="p0", name="pdummy")
            for _ in range(48):
                nc.tensor.matmul(pdummy[0:32, 0:32], wu[:, 0:32], wu[:, 0:32],
                                 start=True, stop=True)
            for _ in range(5):
                nc.tensor.matmul(pdummy[:], wu[:, 0:128], wu[:],
                                 start=True, stop=True)

            # --- DMA prologue, ordered by first PE use, with as few DMAs as
            # possible early (each costs ~625ns of serial HWDGE time). ---
            c0 = CHUNKS[0]
            wt = cp.tile([128, JX * 2 * G3], f8, tag="w")
            nc.sync.dma_start(wt[:, 0:4 * G3], w8d.ap()[:, 0:4 * G3])
            x0 = xp.tile([128, 8 * c0], u8, tag="x8c0")
            nc.sync.dma_start(x0[:], sAd.ap()[:, 0:8 * c0])
            nc.sync.dma_start(wt[:, 4 * G3:8 * G3], w8d.ap()[:, 4 * G3:8 * G3])
            misc = cp.tile([128, MISC], u8, tag="misc")
            nc.sync.dma_start(misc[:], miscd.ap())
            wht = misc[:, 0:2 * G3].bitcast(f8)
            wrt = misc[:, 2 * G3:2 * G3 + JX * 2 * BS].bitcast(f8)
            bt = misc[:, 2 * G3 + JX * 2 * BS:MISC].bitcast(f32)
            hp0 = xp.tile([128, 6 * c0], u8, tag="hpc0")   # h8|h16 block
            nc.sync.dma_start(hp0[:], sAd.ap()[:, 8 * c0:14 * c0])
            xr0 = xrp.tile([128, 8 * c0], f8, tag="xr8c0")
            nc.sync.dma_start(xr0[:], xr8d.ap()[:, 0:8 * c0])
            # Remaining chunks: the [x8] and [h8|h16] halves of the
            # stream block DMA separately so the next chunk's T1 can start
            # as soon as its x8 half lands; xr8 follows (consumed
            # mid-chunk).
            sA_sb, xrc_sb = {}, {}
            cstart = c0
            for c in range(1, len(CHUNKS)):
                cw = CHUNKS[c]
                sc = xp.tile([128, 14 * cw], u8, tag=f"sAc{c}")
                nc.sync.dma_start(sc[:, 0:8 * cw],
                                  sAd.ap()[:, 14 * cstart:14 * cstart + 8 * cw])
                nc.sync.dma_start(sc[:, 8 * cw:14 * cw],
                                  sAd.ap()[:, 14 * cstart + 8 * cw:14 * (cstart + cw)])
                sA_sb[c] = sc
                xrc = xrp.tile([128, 8 * cw], f8, tag=f"xr8c{c}")
                nc.sync.dma_start(xrc[:], xr8d.ap()[:, 8 * cstart:8 * (cstart + cw)])
                xrc_sb[c] = xrc
                cstart += cw

            def wap(j, gt):      # stationary [128, 2, 128] for gate-tile gt
                return (wt[:, j * 2 * G3:(j + 1) * 2 * G3]
                        .rearrange("p (k g) -> p k g", k=2)
                        [:, :, gt * 128:(gt + 1) * 128])

            def wrap_(j, t_):    # W-residual stationary, n-gate tile t_
                return (wrt[:, j * 2 * BS:(j + 1) * 2 * BS]
                        .rearrange("p (k g) -> p k g", k=2)
                        [:, :, t_ * 128:(t_ + 1) * 128])

            def whap(gt):        # hidden stationary
                return (wht.rearrange("p (k g) -> p k g", k=2)
                        [:, :, gt * 128:(gt + 1) * 128])

            cstart = 0
            for c, cw in enumerate(CHUNKS):
                last = (c == len(CHUNKS) - 1)
                if c == 0:
                    def xap(j, cw=cw):
                        return (x0[:, j * 2 * cw:(j + 1) * 2 * cw]
                                .bitcast(f8)
                                .rearrange("p (k b) -> p k b", k=2))
                    hblk = hp0[:]
                else:
                    def xap(j, cw=cw, c=c):
                        return (sA_sb[c][:, j * 2 * cw:(j + 1) * 2 * cw]
                                .bitcast(f8)
                                .rearrange("p (k b) -> p k b", k=2))
                    hblk = sA_sb[c][:, 8 * cw:14 * cw]

                def xrap(j, cw=cw, c=c):
                    t = xr0 if c == 0 else xrc_sb[c]
                    return (t[:, j * 2 * cw:(j + 1) * 2 * cw]
                            .rearrange("p (k b) -> p k b", k=2))

                h8mov = hblk[:, 0:2 * cw].bitcast(f8).rearrange(
                    "p (k b) -> p k b", k=2)

                p_r = [pp.tile([128, cw], f32, tag=f"p{t_}", name=f"pr{t_}")
                       for t_ in range(ST)]
                p_z = [pp.tile([128, cw], f32, tag=f"p{ST + t_}", name=f"pz{t_}")
                       for t_ in range(ST)]
                p_in = [pp.tile([128, cw], f32, tag=f"p{2 * ST + t_}", name=f"pin{t_}")
                        for t_ in range(ST)]
                p_hn = [pp.tile([128, cw], f32, tag=f"p{3 * ST + t_}", name=f"phn{t_}")
                        for t_ in range(ST)]

                # T1 (x8 @ W8) k-major for r/z; the n-gate T1 comes after the
                # z tails so p_in restarts only once the previous chunk's b2
                # has read it, and z completes mid-chunk (its sigmoid frees
                # the bank before the next chunk needs it). The LAST chunk
                # inverts this: i_n completes early so the slow tanh chain
                # runs under the final z matmuls, and z stops last (only
                # sigmoid -> m -> out remain after the PE finishes).
                # r first: its psums complete by DR10 so the r -> a -> b2 ->
                # tanh chain gets maximum runway.
                for j in range(JX):
                    for t_ in range(ST):
                        nc.tensor.matmul(p_r[t_][:], wap(j, t_), xap(j),
                                         start=(j == 0), stop=False,
                                         perf_mode=DR)
                for t_ in range(ST):
                    nc.tensor.matmul(p_r[t_][:], whap(t_), h8mov,
                                     start=False, stop=True, perf_mode=DR)
                for j in range(JX):
                    for t_ in range(ST):
                        nc.tensor.matmul(p_z[t_][:], wap(j, 2 + t_), xap(j),
                                         start=(j == 0), stop=False,
                                         perf_mode=DR)
                # hn psums (h8 projection; the hr8 residual term measurably
                # doesn't move the end-to-end error, so skip it)
                for t_ in range(ST):
                    nc.tensor.matmul(p_hn[t_][:], whap(4 + t_), h8mov,
                                     start=True, stop=True, perf_mode=DR)

                def z_tails(t_):
                    nc.tensor.matmul(p_z[t_][:], whap(2 + t_), h8mov,
                                     start=False, stop=True, perf_mode=DR)

                def n_t1():
                    for j in range(JX):
                        for t_ in range(ST):
                            nc.tensor.matmul(p_in[t_][:], wap(j, 4 + t_),
                                             xap(j), start=(j == 0),
                                             stop=False, perf_mode=DR)

                def in_tails(t_):
                    for j in range(JX):
                        nc.tensor.matmul(p_in[t_][:], wap(j, 4 + t_), xrap(j),
                                         start=False, stop=False, perf_mode=DR)
                    for j in range(JX):
                        nc.tensor.matmul(p_in[t_][:], wrap_(j, t_), xap(j),
                                         start=False, stop=(j == JX - 1),
                                         perf_mode=DR)

                if not last:
                    for t_ in range(ST):
                        z_tails(t_)
                    n_t1()
                    for t_ in range(ST):
                        in_tails(t_)
                else:
                    n_t1()
                    for t_ in range(ST):
                        in_tails(t_)
                    for t_ in range(ST):
                        z_tails(t_)

                # --- elementwise:  out = n + z*(h - n) ---
                # r/z/n land as fp16 halves of full-width tiles so d/m/out
                # can run as single [128, 2cw] fp16 ops (DVE 2x mode).
                o = op.tile([128, ST * cw], f16, tag=f"o{c}")
                zf = gp.tile([128, ST * cw], f16, tag="zf", name="zf")
                nf = gp.tile([128, ST * cw], f16, tag="nf", name="nf")
                h16f = hblk[:, 2 * cw:6 * cw].bitcast(f16)
                r_t, a_t, b2_t = ({} for _ in range(3))

                def ew_r(t_):
                    r = gp.tile([128, cw], f16, tag=f"r{t_}", name=f"r{t_}")
                    nc.scalar.activation(r[:], p_r[t_][:], ACT.Sigmoid,
                                         bias=bt[:, t_:t_ + 1], scale=INV)
                    r_t[t_] = r

                def ew_z(t_):
                    nc.scalar.activation(zf[:, t_ * cw:(t_ + 1) * cw],
                                         p_z[t_][:], ACT.Sigmoid,
                                         bias=bt[:, 2 + t_:3 + t_], scale=INV)

                def ew_a(t_):
                    a = gp.tile([128, cw], f32, tag=f"a{t_}", name=f"a{t_}")
                    nc.vector.scalar_tensor_tensor(
                        a[:], p_hn[t_][:], bt[:, 6 + t_:7 + t_], r_t[t_][:],
                        ALU.add, ALU.mult)
                    a_t[t_] = a

                def ew_b2(t_):
                    b2 = gp.tile([128, cw], f32, tag=f"b{t_}", name=f"b{t_}")
                    nc.vector.tensor_add(b2[:], a_t[t_][:], p_in[t_][:])
                    b2_t[t_] = b2

                def ew_n(t_):
                    nc.scalar.activation(nf[:, t_ * cw:(t_ + 1) * cw],
                                         b2_t[t_][:], ACT.Tanh,
                                         bias=bt[:, 4 + t_:5 + t_], scale=INV)

                def ew_zc(t_):
                    zc = gp.tile([128, cw], f16, tag=f"zc{t_}", name=f"zc{t_}")
                    nc.scalar.activation(zc[:], p_z[t_][:], ACT.Sigmoid,
                                         bias=bt[:, 8 + t_:9 + t_], scale=-INV)
                    return zc

                cs = slice(cstart, cstart + cw)
                for t_ in range(ST):
                    ew_r(t_)
                if not last:
                    for t_ in range(ST):
                        ew_a(t_)
                    for t_ in range(ST):
                        ew_z(t_)
                    zc1 = ew_zc(1)
                    zh1 = gp.tile([128, cw], f16, tag="zh1", name="zh1")
                    nc.gpsimd.tensor_mul(zh1[:], zf[:, cw:2 * cw],
                                         h16f[:, cw:2 * cw])
                    for t_ in range(ST):
                        ew_b2(t_)
                else:
                    # inverted last chunk: i_n stops first, z last. Emit the
                    # tanh path before the z sigmoids on every queue; both
                    # a's ahead of the b2's (a parked behind a waiting b2
                    # would head-block the FIFO wait queue).
                    ew_a(0)
                    ew_a(1)
                    ew_b2(0)
                    ew_b2(1)
                    for t_ in range(ST):
                        ew_n(t_)
                    for t_ in range(ST):
                        ew_z(t_)
                if not last:
                    for t_ in range(ST):
                        ew_n(t_)
                # per-tile fp16 combine chains: t0 on VectorE (2x mode),
                # t1 on the otherwise-idle Pool for the big mid chunks so
                # the DVE queue stays clear for the next chunk's PSUM
                # drains. The small tail chunks interleave both chains on
                # the fast VectorE.
                if last:
                    sl = [slice(t_ * cw, (t_ + 1) * cw) for t_ in range(ST)]
                    d, m = [], []
                    for t_ in range(ST):
                        d.append(gp.tile([128, cw], f16, tag=f"d{t_}",
                                         name=f"d{t_}"))
                        nc.vector.tensor_sub(d[t_][:], h16f[:, sl[t_]],
                                             nf[:, sl[t_]])
                    for t_ in range(ST):
                        m.append(gp.tile([128, cw], f16, tag=f"m{t_}",
                                         name=f"m{t_}"))
                        nc.vector.tensor_mul(m[t_][:], zf[:, sl[t_]], d[t_][:])
                    for t_ in range(ST):
                        nc.vector.tensor_add(o[:, sl[t_]], nf[:, sl[t_]],
                                             m[t_][:])
                else:
                    # t0's chain on VectorE (2x mode). t1's chain rides the
                    # idle Pool (FIFO wait queues — a long-parked op on DVE
                    # would head-block), in the zc/zh form so only two Pool
                    # ops remain after the tanh lands: out1 = zc1*n1 + zh1.
                    d = gp.tile([128, cw], f16, tag="d0", name="d0")
                    nc.vector.tensor_sub(d[:], h16f[:, 0:cw], nf[:, 0:cw])
                    m = gp.tile([128, cw], f16, tag="m0", name="m0")
                    nc.vector.tensor_mul(m[:], zf[:, 0:cw], d[:])
                    nc.vector.tensor_add(o[:, 0:cw], nf[:, 0:cw], m[:])
                    e1 = gp.tile([128, cw], f16, tag="e1", name="e1")
                    nc.gpsimd.tensor_mul(e1[:], zc1[:], nf[:, cw:2 * cw])
                    nc.gpsimd.tensor_add(o[:, cw:2 * cw], e1[:], zh1[:])
                nc.sync.dma_start(
                    oT.ap().rearrange("(t p) b -> p t b", p=128)[:, :, cs],
                    o[:].rearrange("p (t c) -> p t c", t=ST))
                cstart += cw

    nc.compile()
    return nc


def _get_nc():
    global _cached
    if _cached is None:
        _cached = _build()
    return _cached


def kernel(input, hidden, W_ih, W_hh, b_ih, b_hh):
    input = np.asarray(input, dtype=np.float32)
    hidden = np.asarray(hidden, dtype=np.float32)
    W_ih = np.asarray(W_ih, dtype=np.float32)
    W_hh = np.asarray(W_hh, dtype=np.float32)
    b_ih = np.asarray(b_ih, dtype=np.float32)
    b_hh = np.asarray(b_hh, dtype=np.float32)

    nc = _get_nc()
    from concourse.bass_utils import run_bass_kernel_spmd

    # input-side quantization (shared by all blocks)
    X = input.T * SX                              # [1024, 2048]
    x8 = X.astype(E4)
    xr8 = (X - x8.astype(np.float32)).astype(E4)
    x8v = x8.reshape(8, 128, BATCH).view(np.uint8)  # [ktile, p, b]

    def pack(planes):
        """planes: list of [np, 128, width] u8 arrays sharing the CHUNKS
        column split (width = scale*BATCH)."""
        blocks = []
        off = 0
        for cw in CHUNKS:
            blk = []
            for pl in planes:
                s = pl.shape[2] // BATCH
                blk.append(pl[:, :, s * off:s * (off + cw)]
                           .transpose(1, 0, 2).reshape(128, -1))
            blocks.append(np.concatenate(blk, axis=1))
            off += cw
        return np.ascontiguousarray(np.concatenate(blocks, axis=1))

    xr8p = pack([xr8.reshape(8, 128, BATCH).view(np.uint8)]).view(E4)

    in_maps = []
    for n in range(NUM_BLOCKS):
        Wi = W_ih[n].T * SW                       # [1024, 768]
        W8 = Wi.astype(E4)
        WR8 = (Wi - W8.astype(np.float32))[:, 2 * BS:].astype(E4)  # n gate
        w8p = np.ascontiguousarray(
            W8.reshape(JX, 2, 128, G3).transpose(2, 0, 1, 3).reshape(128, JX * 2 * G3))
        wr8p = WR8.reshape(JX, 2, 128, BS).transpose(2, 0, 1, 3).reshape(128, JX * 2 * BS)
        Wh = W_hh[n].T * SW                       # [256, 768]
        wh8p = Wh.astype(E4).reshape(2, 128, G3).transpose(1, 0, 2).reshape(128, 2 * G3)

        Hb = hidden[:, n * BS:(n + 1) * BS].T     # [256, 2048]
        Hs = Hb * SX
        h8 = Hs.astype(E4)
        hr8 = (Hs - h8.astype(np.float32)).astype(E4)
        h16 = np.ascontiguousarray(Hb.astype(np.float16).reshape(2, 128, BATCH))
        sA = pack([x8v,
                   h8.reshape(2, 128, BATCH).view(np.uint8),
                   h16.view(np.uint8).reshape(2, 128, 2 * BATCH)])

        brz = b_ih[n, :2 * BS] + b_hh[n, :2 * BS]          # r,z: fused bias
        bias = np.concatenate([
            brz[:BS].reshape(2, 128).T,                    # br0 br1
            brz[BS:].reshape(2, 128).T,                    # bz0 bz1
            b_ih[n, 2 * BS:].reshape(2, 128).T,            # bin0 bin1
            (b_hh[n, 2 * BS:] * Q).reshape(2, 128).T,      # bhnQ0 bhnQ1
            -brz[BS:].reshape(2, 128).T,                   # bnegz0 bnegz1
        ], axis=1).astype(np.float32)
        misc = np.concatenate([
            np.ascontiguousarray(wh8p).view(np.uint8),
            np.ascontiguousarray(wr8p).view(np.uint8),
            np.ascontiguousarray(bias).view(np.uint8).reshape(128, 40),
        ], axis=1)

        in_maps.append({
            "sA": sA,
            "xr8": xr8p,
            "w8": w8p,
            "misc": np.ascontiguousarray(misc),
        })

    res = run_bass_kernel_spmd(nc, in_maps, core_ids=list(range(NUM_BLOCKS)))
    out = np.empty((BATCH, HIDDEN_DIM), dtype=np.float32)
    for n in range(NUM_BLOCKS):
        out[:, n * BS:(n + 1) * BS] = res.results[n]["oT"].T.astype(np.float32)
    return out


# revision 62
# speedup vs baseline: 1.0003x; 1.0003x over previous
"""BlockGRU Trainium2 kernel — fp8 DoubleRow edition.

Block-diagonal GRU cell: 8 independent blocks (block_size 256), batch 2048,
input_dim 1024. Sharded one block per NeuronCore (8 cores).

All matmuls run in fp8 e4m3 with MatmulPerfMode.DoubleRow (0.5 cycles per
output row, two 128-deep k-tiles per instruction -> 4x the fp16 PE rate in
the cost model). Precision is recovered with residual ("split hi/lo")
correction terms, applied only where the end-to-end error needs them:

  gi = x8@W8 (+ xr8@W8 + x8@WR8 on the n gate)
  gh = h8@Wh8

where x8 = e4m3(16*x), xr8 = e4m3(16*x - x8), W8 = e4m3(1024*W),
WR8 = e4m3(1024*W - W8), h8 = e4m3(16*h). All PSUM pre-activations share
one scale 16384, folded into the ScalarE activation `scale` operand.
Measured end-to-end rel-L2 error vs the fp32 reference: 1.632e-2
(gate: 2e-2; inputs are a fixed seed, so this is deterministic — the
numpy predictor in the dev transcript matches the kernel to ~1e-5).
Output is fp16, upcast on the host.

Elementwise pipeline (per state-tile): r and z sigmoids and the n tanh on
ScalarE (PSUM reads, fp16 outputs); the r-gating stt and the i_n add on
VectorE in fp32 (PSUM operands, magnitude up to ~1e5 — fp16 would
overflow); the output combine out = n + z*(h16 - n) in all-fp16
tensor_tensor ops, which the DVE cost model runs at 2x (2-byte packed
operands). h16 rides the per-chunk input stream. For mid chunks the
t0 combine runs on VectorE and the t1 combine on the otherwise-idle Pool
in the equivalent zc*n + z*h form (zc = sigmoid(-pre), so only two Pool
ops remain once the tanh lands); engine wait queues release in FIFO
order, so long-parked ops must never sit ahead of soon-needed ones on the
same queue. Output DMAs ride the otherwise-idle SP queue.

PE order per chunk completes psums in elementwise consumption order
(r -> hn -> z -> i_n) and starts the n-gate T1 late so the previous
chunk's b2 has drained PSUM; the last chunk inverts to r -> z-T1 -> hn ->
i_n -> z-tails so only sigmoid -> m -> out remain after the final matmul.
Input DMAs are merged and ordered so arrival tracks first-use (the serial
HWDGE costs ~625ns per DMA), and the warm-up is sized to keep the PE busy
until the first weights arrive (an idle PE resets the p-state ramp).
"""

import sys

if "/opt/trn_rl_repo" not in sys.path:
    sys.path.insert(0, "/opt/trn_rl_repo")

import numpy as np
import ml_dtypes

INPUT_DIM = 1024
HIDDEN_DIM = 2048
NUM_BLOCKS = 8
BS = HIDDEN_DIM // NUM_BLOCKS  # 256
G3 = 3 * BS                    # 768
BATCH = 2048
CHUNKS = [256, 512, 512, 512, 256]
JX = 4                         # input-side k-pairs (8 k-tiles, DoubleRow'd)
ST = 2                         # state partition-tiles per block
SX = 16.0                      # x / h quantization scale
SW = 1024.0                    # weight quantization scale
Q = SX * SW                    # psum pre-activation scale
INV = 1.0 / Q
MISC = 2 * G3 + JX * 2 * BS + 40   # wh | wr | bias bytes per partition

E4 = ml_dtypes.float8_e4m3

_cached = None


def _build():
    import concourse.tile as tile
    import concourse.mybir as mybir
    from concourse import bacc

    f32 = mybir.dt.float32
    f16 = mybir.dt.float16
    f8 = mybir.dt.float8e4
    u8 = mybir.dt.uint8
    ALU = mybir.AluOpType
    ACT = mybir.ActivationFunctionType
    DR = mybir.MatmulPerfMode.DoubleRow

    nc = bacc.Bacc("TRN2", target_bir_lowering=False, debug=False, num_devices=8)

    # DRAM tensors. Free-dim layouts are pre-packed on the host so every DMA
    # lands >=512B-contiguous runs (fp8 would otherwise pay the 2x
    # small-element DMA penalty). sA carries per-chunk
    # [x8 (8cw) | h8 (2cw) | h16 (4cw bytes)] blocks; xr8 streams
    # separately (it is consumed mid-chunk).
    sAd = nc.dram_tensor("sA", [128, 14 * BATCH], u8, kind="ExternalInput")
    xr8d = nc.dram_tensor("xr8", [128, 8 * BATCH], f8, kind="ExternalInput")
    w8d = nc.dram_tensor("w8", [128, JX * 2 * G3], f8, kind="ExternalInput")
    miscd = nc.dram_tensor("misc", [128, MISC], u8, kind="ExternalInput")
    oT = nc.dram_tensor("oT", [BS, BATCH], f16, kind="ExternalOutput")

    with tile.TileContext(nc) as tc:
        with (
            tc.tile_pool(name="const", bufs=1) as cp,
            tc.tile_pool(name="xin", bufs=1) as xp,
            tc.tile_pool(name="xrin", bufs=1) as xrp,
            tc.tile_pool(name="gates", bufs=2) as gp,
            tc.tile_pool(name="outs", bufs=1) as op,
            tc.tile_pool(name="psum", bufs=1, space="PSUM") as pp,
        ):
            # PE warm-up: matmuls on a zeroed tile while the prefill DMA
            # runs. Sized to keep the PE busy until the first weights/x
            # arrive (~5.8us): an idle PE resets the p-state ramp and the
            # first 3us of real matmuls would run at half clock.
            wu = cp.tile([128, 512], f16, tag="wu")
            nc.vector.memset(wu[:], 0.0)
            pdummy = pp.tile([128, 512], f32, tag="p0", name="pdummy")
            for _ in range(48):
                nc.tensor.matmul(pdummy[0:32, 0:32], wu[:, 0:32], wu[:, 0:32],
                                 start=True, stop=True)
            for _ in range(5):
                nc.tensor.matmul(pdummy[:], wu[:, 0:128], wu[:],
                                 start=True, stop=True)

            # --- DMA prologue, ordered by first PE use, with as few DMAs as
            # possible early (each costs ~625ns of serial HWDGE time). ---
            c0 = CHUNKS[0]
            wt = cp.tile([128, JX * 2 * G3], f8, tag="w")
            nc.sync.dma_start(wt[:, 0:4 * G3], w8d.ap()[:, 0:4 * G3])
            x0 = xp.tile([128, 8 * c0], u8, tag="x8c0")
            nc.sync.dma_start(x0[:], sAd.ap()[:, 0:8 * c0])
            nc.sync.dma_start(wt[:, 4 * G3:8 * G3], w8d.ap()[:, 4 * G3:8 * G3])
            misc = cp.tile([128, MISC], u8, tag="misc")
            nc.sync.dma_start(misc[:], miscd.ap())
            wht = misc[:, 0:2 * G3].bitcast(f8)
            wrt = misc[:, 2 * G3:2 * G3 + JX * 2 * BS].bitcast(f8)
            bt = misc[:, 2 * G3 + JX * 2 * BS:MISC].bitcast(f32)
            hp0 = xp.tile([128, 6 * c0], u8, tag="hpc0")   # h8|h16 block
            nc.sync.dma_start(hp0[:], sAd.ap()[:, 8 * c0:14 * c0])
            xr0 = xrp.tile([128, 8 * c0], f8, tag="xr8c0")
            nc.sync.dma_start(xr0[:], xr8d.ap()[:, 0:8 * c0])
            # Remaining chunks: the [x8] and [h8|h16] halves of the
            # stream block DMA separately so the next chunk's T1 can start
            # as soon as its x8 half lands; xr8 follows (consumed
            # mid-chunk).
            sA_sb, xrc_sb = {}, {}
            cstart = c0
            for c in range(1, len(CHUNKS)):
                cw = CHUNKS[c]
                sc = xp.tile([128, 14 * cw], u8, tag=f"sAc{c}")
                nc.sync.dma_start(sc[:, 0:8 * cw],
                                  sAd.ap()[:, 14 * cstart:14 * cstart + 8 * cw])
                nc.sync.dma_start(sc[:, 8 * cw:14 * cw],
                                  sAd.ap()[:, 14 * cstart + 8 * cw:14 * (cstart + cw)])
                sA_sb[c] = sc
                xrc = xrp.tile([128, 8 * cw], f8, tag=f"xr8c{c}")
                nc.sync.dma_start(xrc[:], xr8d.ap()[:, 8 * cstart:8 * (cstart + cw)])
                xrc_sb[c] = xrc
                cstart += cw

            def wap(j, gt):      # stationary [128, 2, 128] for gate-tile gt
                return (wt[:, j * 2 * G3:(j + 1) * 2 * G3]
                        .rearrange("p (k g) -> p k g", k=2)
                        [:, :, gt * 128:(gt + 1) * 128])

            def wrap_(j, t_):    # W-residual stationary, n-gate tile t_
                return (wrt[:, j * 2 * BS:(j + 1) * 2 * BS]
                        .rearrange("p (k g) -> p k g", k=2)
                        [:, :, t_ * 128:(t_ + 1) * 128])

            def whap(gt):        # hidden stationary
                return (wht.rearrange("p (k g) -> p k g", k=2)
                        [:, :, gt * 128:(gt + 1) * 128])

            cstart = 0
            for c, cw in enumerate(CHUNKS):
                last = (c == len(CHUNKS) - 1)
                if c == 0:
                    def xap(j, cw=cw):
                        return (x0[:, j * 2 * cw:(j + 1) * 2 * cw]
                                .bitcast(f8)
                                .rearrange("p (k b) -> p k b", k=2))
                    hblk = hp0[:]
                else:
                    def xap(j, cw=cw, c=c):
                        return (sA_sb[c][:, j * 2 * cw:(j + 1) * 2 * cw]
                                .bitcast(f8)
                                .rearrange("p (k b) -> p k b", k=2))
                    hblk = sA_sb[c][:, 8 * cw:14 * cw]

                def xrap(j, cw=cw, c=c):
                    t = xr0 if c == 0 else xrc_sb[c]
                    return (t[:, j * 2 * cw:(j + 1) * 2 * cw]
                            .rearrange("p (k b) -> p k b", k=2))

                h8mov = hblk[:, 0:2 * cw].bitcast(f8).rearrange(
                    "p (k b) -> p k b", k=2)

                p_r = [pp.tile([128, cw], f32, tag=f"p{t_}", name=f"pr{t_}")
                       for t_ in range(ST)]
                p_z = [pp.tile([128, cw], f32, tag=f"p{ST + t_}", name=f"pz{t_}")
                       for t_ in range(ST)]
                p_in = [pp.tile([128, cw], f32, tag=f"p{2 * ST + t_}", name=f"pin{t_}")
                        for t_ in range(ST)]
                p_hn = [pp.tile([128, cw], f32, tag=f"p{3 * ST + t_}", name=f"phn{t_}")
                        for t_ in range(ST)]

                # T1 (x8 @ W8) k-major for r/z; the n-gate T1 comes after the
                # z tails so p_in restarts only once the previous chunk's b2
                # has read it, and z completes mid-chunk (its sigmoid frees
                # the bank before the next chunk needs it). The LAST chunk
                # inverts this: i_n completes early so the slow tanh chain
                # runs under the final z matmuls, and z stops last (only
                # sigmoid -> m -> out remain after the PE finishes).
                # r first: its psums complete by DR10 so the r -> a -> b2 ->
                # tanh chain gets maximum runway.
                for j in range(JX):
                    for t_ in range(ST):
                        nc.tensor.matmul(p_r[t_][:], wap(j, t_), xap(j),
                                         start=(j == 0), stop=False,
                                         perf_mode=DR)
                for t_ in range(ST):
                    nc.tensor.matmul(p_r[t_][:], whap(t_), h8mov,
                                     start=False, stop=True, perf_mode=DR)
                for j in range(JX):
                    for t_ in range(ST):
                        nc.tensor.matmul(p_z[t_][:], wap(j, 2 + t_), xap(j),
                                         start=(j == 0), stop=False,
                                         perf_mode=DR)
                # hn psums (h8 projection; the hr8 residual term measurably
                # doesn't move the end-to-end error, so skip it)
                for t_ in range(ST):
                    nc.tensor.matmul(p_hn[t_][:], whap(4 + t_), h8mov,
                                     start=True, stop=True, perf_mode=DR)

                def z_tails(t_):
                    nc.tensor.matmul(p_z[t_][:], whap(2 + t_), h8mov,
                                     start=False, stop=True, perf_mode=DR)

                def n_t1():
                    for j in range(JX):
                        for t_ in range(ST):
                            nc.tensor.matmul(p_in[t_][:], wap(j, 4 + t_),
                                             xap(j), start=(j == 0),
                                             stop=False, perf_mode=DR)

                def in_tails(t_):
                    for j in range(JX):
                        nc.tensor.matmul(p_in[t_][:], wap(j, 4 + t_), xrap(j),
                                         start=False, stop=False, perf_mode=DR)
                    for j in range(JX):
                        nc.tensor.matmul(p_in[t_][:], wrap_(j, t_), xap(j),
                                         start=False, stop=(j == JX - 1),
                                         perf_mode=DR)

                if not last:
                    for t_ in range(ST):
                        z_tails(t_)
                    n_t1()
                    for t_ in range(ST):
                        in_tails(t_)
                else:
                    n_t1()
                    for t_ in range(ST):
                        in_tails(t_)
                    for t_ in range(ST):
                        z_tails(t_)

                # --- elementwise:  out = n + z*(h - n) ---
                # r/z/n land as fp16 halves of full-width tiles so d/m/out
                # can run as single [128, 2cw] fp16 ops (DVE 2x mode).
                o = op.tile([128, ST * cw], f16, tag=f"o{c}")
                zf = gp.tile([128, ST * cw], f16, tag="zf", name="zf")
                nf = gp.tile([128, ST * cw], f16, tag="nf", name="nf")
                h16f = hblk[:, 2 * cw:6 * cw].bitcast(f16)
                r_t, a_t, b2_t = ({} for _ in range(3))

                def ew_r(t_):
                    r = gp.tile([128, cw], f16, tag=f"r{t_}", name=f"r{t_}")
                    nc.scalar.activation(r[:], p_r[t_][:], ACT.Sigmoid,
                                         bias=bt[:, t_:t_ + 1], scale=INV)
                    r_t[t_] = r

                def ew_z(t_):
                    nc.scalar.activation(zf[:, t_ * cw:(t_ + 1) * cw],
                                         p_z[t_][:], ACT.Sigmoid,
                                         bias=bt[:, 2 + t_:3 + t_], scale=INV)

                def ew_a(t_):
                    a = gp.tile([128, cw], f32, tag=f"a{t_}", name=f"a{t_}")
                    nc.vector.scalar_tensor_tensor(
                        a[:], p_hn[t_][:], bt[:, 6 + t_:7 + t_], r_t[t_][:],
                        ALU.add, ALU.mult)
                    a_t[t_] = a

                def ew_b2(t_):
                    b2 = gp.tile([128, cw], f32, tag=f"b{t_}", name=f"b{t_}")
                    nc.vector.tensor_add(b2[:], a_t[t_][:], p_in[t_][:])
                    b2_t[t_] = b2

                def ew_n(t_):
                    nc.scalar.activation(nf[:, t_ * cw:(t_ + 1) * cw],
                                         b2_t[t_][:], ACT.Tanh,
                                         bias=bt[:, 4 + t_:5 + t_], scale=INV)

                def ew_zc(t_):
                    zc = gp.tile([128, cw], f16, tag=f"zc{t_}", name=f"zc{t_}")
                    nc.scalar.activation(zc[:], p_z[t_][:], ACT.Sigmoid,
                                         bias=bt[:, 8 + t_:9 + t_], scale=-INV)
                    return zc

                cs = slice(cstart, cstart + cw)
                for t_ in range(ST):
                    ew_r(t_)
                if not last:
                    for t_ in range(ST):
                        ew_a(t_)
                    for t_ in range(ST):
                        ew_z(t_)
                    zc1 = ew_zc(1)
                    zh1 = gp.tile([128, cw], f16, tag="zh1", name="zh1")
                    nc.gpsimd.tensor_mul(zh1[:], zf[:, cw:2 * cw],
                                         h16f[:, cw:2 * cw])
                    for t_ in range(ST):
                        ew_b2(t_)
                else:
                    # inverted last chunk: i_n stops first, z last. Emit the
                    # tanh path before the z sigmoids on every queue.
                    ew_a(0)
                    ew_b2(0)
                    ew_a(1)
                    ew_b2(1)
                    for t_ in range(ST):
                        ew_n(t_)
                    for t_ in range(ST):
                        ew_z(t_)
                if not last:
                    for t_ in range(ST):
                        ew_n(t_)
                # per-tile fp16 combine chains: t0 on VectorE (2x mode),
                # t1 on the otherwise-idle Pool for the big mid chunks so
                # the DVE queue stays clear for the next chunk's PSUM
                # drains. The small tail chunks interleave both chains on
                # the fast VectorE.
                if last:
                    sl = [slice(t_ * cw, (t_ + 1) * cw) for t_ in range(ST)]
                    d, m = [], []
                    for t_ in range(ST):
                        d.append(gp.tile([128, cw], f16, tag=f"d{t_}",
                                         name=f"d{t_}"))
                        nc.vector.tensor_sub(d[t_][:], h16f[:, sl[t_]],
                                             nf[:, sl[t_]])
                    for t_ in range(ST):
                        m.append(gp.tile([128, cw], f16, tag=f"m{t_}",
                                         name=f"m{t_}"))
                        nc.vector.tensor_mul(m[t_][:], zf[:, sl[t_]], d[t_][:])
                    for t_ in range(ST):
                        nc.vector.tensor_add(o[:, sl[t_]], nf[:, sl[t_]],
                                             m[t_][:])
                else:
                    # t0's chain on VectorE (2x mode). t1's chain rides the
                    # idle Pool (FIFO wait queues — a long-parked op on DVE
                    # would head-block), in the zc/zh form so only two Pool
                    # ops remain after the tanh lands: out1 = zc1*n1 + zh1.
                    d = gp.tile([128, cw], f16, tag="d0", name="d0")
                    nc.vector.tensor_sub(d[:], h16f[:, 0:cw], nf[:, 0:cw])
                    m = gp.tile([128, cw], f16, tag="m0", name="m0")
                    nc.vector.tensor_mul(m[:], zf[:, 0:cw], d[:])
                    nc.vector.tensor_add(o[:, 0:cw], nf[:, 0:cw], m[:])
                    e1 = gp.tile([128, cw], f16, tag="e1", name="e1")
                    nc.gpsimd.tensor_mul(e1[:], zc1[:], nf[:, cw:2 * cw])
                    nc.gpsimd.tensor_add(o[:, cw:2 * cw], e1[:], zh1[:])
                nc.sync.dma_start(
                    oT.ap().rearrange("(t p) b -> p t b", p=128)[:, :, cs],
                    o[:].rearrange("p (t c) -> p t c", t=ST))
                cstart += cw

    nc.compile()
    return nc


def _get_nc():
    global _cached
    if _cached is None:
        _cached = _build()
    return _cached


def kernel(input, hidden, W_ih, W_hh, b_ih, b_hh):
    input = np.asarray(input, dtype=np.float32)
    hidden = np.asarray(hidden, dtype=np.float32)
    W_ih = np.asarray(W_ih, dtype=np.float32)
    W_hh = np.asarray(W_hh, dtype=np.float32)
    b_ih = np.asarray(b_ih, dtype=np.float32)
    b_hh = np.asarray(b_hh, dtype=np.float32)

    nc = _get_nc()
    from concourse.bass_utils import run_bass_kernel_spmd

    # input-side quantization (shared by all blocks)
    X = input.T * SX                              # [1024, 2048]
    x8 = X.astype(E4)
    xr8 = (X - x8.astype(np.float32)).astype(E4)
    x8v = x8.reshape(8, 128, BATCH).view(np.uint8)  # [ktile, p, b]

    def pack(planes):
        """planes: list of [np, 128, width] u8 arrays sharing the CHUNKS
        column split (width = scale*BATCH)."""
        blocks = []
        off = 0
        for cw in CHUNKS:
            blk = []
            for pl in planes:
                s = pl.shape[2] // BATCH
                blk.append(pl[:, :, s * off:s * (off + cw)]
                           .transpose(1, 0, 2).reshape(128, -1))
            blocks.append(np.concatenate(blk, axis=1))
            off += cw
        return np.ascontiguousarray(np.concatenate(blocks, axis=1))

    xr8p = pack([xr8.reshape(8, 128, BATCH).view(np.uint8)]).view(E4)

    in_maps = []
    for n in range(NUM_BLOCKS):
        Wi = W_ih[n].T * SW                       # [1024, 768]
        W8 = Wi.astype(E4)
        WR8 = (Wi - W8.astype(np.float32))[:, 2 * BS:].astype(E4)  # n gate
        w8p = np.ascontiguousarray(
            W8.reshape(JX, 2, 128, G3).transpose(2, 0, 1, 3).reshape(128, JX * 2 * G3))
        wr8p = WR8.reshape(JX, 2, 128, BS).transpose(2, 0, 1, 3).reshape(128, JX * 2 * BS)
        Wh = W_hh[n].T * SW                       # [256, 768]
        wh8p = Wh.astype(E4).reshape(2, 128, G3).transpose(1, 0, 2).reshape(128, 2 * G3)

        Hb = hidden[:, n * BS:(n + 1) * BS].T     # [256, 2048]
        Hs = Hb * SX
        h8 = Hs.astype(E4)
        hr8 = (Hs - h8.astype(np.float32)).astype(E4)
        h16 = np.ascontiguousarray(Hb.astype(np.float16).reshape(2, 128, BATCH))
        sA = pack([x8v,
                   h8.reshape(2, 128, BATCH).view(np.uint8),
                   h16.view(np.uint8).reshape(2, 128, 2 * BATCH)])

        brz = b_ih[n, :2 * BS] + b_hh[n, :2 * BS]          # r,z: fused bias
        bias = np.concatenate([
            brz[:BS].reshape(2, 128).T,                    # br0 br1
            brz[BS:].reshape(2, 128).T,                    # bz0 bz1
            b_ih[n, 2 * BS:].reshape(2, 128).T,            # bin0 bin1
            (b_hh[n, 2 * BS:] * Q).reshape(2, 128).T,      # bhnQ0 bhnQ1
            -brz[BS:].reshape(2, 128).T,                   # bnegz0 bnegz1
        ], axis=1).astype(np.float32)
        misc = np.concatenate([
            np.ascontiguousarray(wh8p).view(np.uint8),
            np.ascontiguousarray(wr8p).view(np.uint8),
            np.ascontiguousarray(bias).view(np.uint8).reshape(128, 40),
        ], axis=1)

        in_maps.append({
            "sA": sA,
            "xr8": xr8p,
            "w8": w8p,
            "misc": np.ascontiguousarray(misc),
        })

    res = run_bass_kernel_spmd(nc, in_maps, core_ids=list(range(NUM_BLOCKS)))
    out = np.empty((BATCH, HIDDEN_DIM), dtype=np.float32)
    for n in range(NUM_BLOCKS):
        out[:, n * BS:(n + 1) * BS] = res.results[n]["oT"].T.astype(np.float32)
    return out
